# revision 19
# baseline (speedup 1.0000x reference)
"""Trainium2 Bass kernel for nn_AVQ (vq_codebook): additive VQ beam search.

Data-parallel over N: each of the 8 NeuronCores processes 512 samples with the
full [4096, 256] codebook replicated. The beam search (4 stages x 1024
codebooks, beam 4) runs fully on-device per core:

  - selection scores  v = 2*r.cb - cb2  (fine-grid: the ||r||^2 row-constant
    does not affect per-row ranking, and avoiding it kills fp32 tie plateaus)
    computed by TensorE as K=257 chunked matmuls (stationary = [-r^T; ones],
    moving = [-2*cb^T; -cb2]); stage banning via per-partition ACT bias -1e9
  - per-beam top-4 over 4096 via DVE max8 + max_index
  - true-remainder scores at the 16 candidates per sample as deltas vs the
    selection scores: dstrip = -2 * dot(dm_row, cb[cand]) with cb rows
    fetched by per-partition indirect DMA gathers, dots on GPSIMD (STT accum)
  - beam pruning via packed int32 keys  (bits(d)<<4 | j)  -> exact, tie-free
    selection with jax-compatible lowest-index tie-breaking
  - beam state updates via indirect row gathers (codebook rows and the
    beam-sum rows bounced through DRAM) + PE transposes for the stationaries

Outputs per core: nsvq [512, 256] f32 and the winning tuple best [512, 4] i32.
Host glue: shard/concat, scatter `used`, and (train_mode=0) the trivial
quantized = cb[best].sum(1).
"""
import os
import sys

sys.path.insert(0, "/opt/trn_rl_repo")

from contextlib import ExitStack

import numpy as np

from concourse import bass, mybir
from concourse import tile as tile_mod
from concourse import bass_utils
from concourse.masks import make_identity
from bass_rust import add_dep_helper

f32 = mybir.dt.float32
i32 = mybir.dt.int32
u32 = mybir.dt.uint32

S, KCB, W, D = 4, 1024, 4, 256
NTOT = 4096
NSH = 512          # samples per core
G = 4              # groups of 128 samples
NC = 8
BIG = 1.0e9
EXP_OFF = 0x43000000   # bits(128.0); d clamped to [128, 512)
KNEG_C = 0x40000000

AF = mybir.ActivationFunctionType
ALU = mybir.AluOpType
AX = mybir.AxisListType

MM_DT = f32   # switch to mybir.dt.float32r after numerics check
DEBUG_TAPS = False


def _mm(ap):
    if MM_DT == f32:
        return ap
    return ap.bitcast(MM_DT)


def build_nc():
    nc = bass.Bass(target_bir_lowering=False)

    d_xT = nc.dram_tensor("xTneg", [256, NSH], f32, kind="ExternalInput")
    d_x = nc.dram_tensor("xsm", [NSH, D], f32, kind="ExternalInput")
    d_rv = nc.dram_tensor("rvsm", [NSH, D], f32, kind="ExternalInput")
    d_cbT = nc.dram_tensor("cbTm2", [256, NTOT], f32, kind="ExternalInput")
    d_cb2 = nc.dram_tensor("cb2neg", [1, NTOT], f32, kind="ExternalInput")
    d_cbr = nc.dram_tensor("cbrows", [NTOT, D], f32, kind="ExternalInput")
    d_nsvq = nc.dram_tensor("nsvq", [NSH, D], f32, kind="ExternalOutput")
    d_best = nc.dram_tensor("best", [NSH, S], i32, kind="ExternalOutput")
    d_ss0 = nc.dram_tensor("ssum0", [NSH * 4, D], f32)
    d_ss1 = nc.dram_tensor("ssum1", [NSH * 4, D], f32)
    dbg = {}
    if DEBUG_TAPS:
        for it in range(3):
            dbg[f"dbg_cand{it}"] = nc.dram_tensor(
                f"dbg_cand{it}", [128, G, 16], f32, kind="ExternalOutput")
            dbg[f"dbg_jsel{it}"] = nc.dram_tensor(
                f"dbg_jsel{it}", [128, G, 4], i32, kind="ExternalOutput")
            dbg[f"dbg_d16_{it}"] = nc.dram_tensor(
                f"dbg_d16_{it}", [128, G, 16], f32, kind="ExternalOutput")
        dbg["dbg_scand"] = nc.dram_tensor(
            "dbg_scand", [128, G, 4], i32, kind="ExternalOutput")

    with tile_mod.TileContext(nc) as tc, ExitStack() as ctx:
        build_kernel(nc, tc, ctx, d_xT, d_x, d_rv, d_cbT, d_cb2, d_cbr,
                     d_nsvq, d_best, [d_ss0, d_ss1], dbg)
    return nc


def build_kernel(nc, tc, ctx, d_xT, d_x, d_rv, d_cbT, d_cb2, d_cbr,
                 d_nsvq, d_best, d_ssum, dbg):
    singles = ctx.enter_context(tc.tile_pool(name="singles", bufs=1))
    state = ctx.enter_context(tc.tile_pool(name="state", bufs=2))
    mstate = ctx.enter_context(tc.tile_pool(name="mstate", bufs=1))
    vpool = ctx.enter_context(tc.tile_pool(name="vtiles", bufs=2))
    rows = ctx.enter_context(tc.tile_pool(name="rows", bufs=3))
    small = ctx.enter_context(tc.tile_pool(name="small", bufs=4))
    psc = ctx.enter_context(tc.tile_pool(name="psc", bufs=5, space="PSUM"))
    ptr = ctx.enter_context(tc.tile_pool(name="ptr", bufs=2, space="PSUM"))

    # ---- residents ----
    cbA = singles.tile([128, NTOT], f32)
    cbB = singles.tile([128, NTOT], f32)
    cbC = singles.tile([1, NTOT], f32)
    xA = singles.tile([128, NSH], f32)
    xB = singles.tile([128, NSH], f32)
    nc.sync.dma_start(out=cbA, in_=d_cbT[0:128, :])
    nc.sync.dma_start(out=cbB, in_=d_cbT[128:256, :])
    nc.sync.dma_start(out=cbC, in_=d_cb2[:, :])
    nc.sync.dma_start(out=xA, in_=d_xT[0:128, :])
    nc.sync.dma_start(out=xB, in_=d_xT[128:256, :])

    def pe_touch(ap):
        """Junk bf16 ldweights that makes PE observe `ap`'s producer sem, so
        later S3_LW instructions (fp32 self-loading matmuls / transposes)
        carry at most one sync wait each (hardware slot limit)."""
        return nc.tensor.ldweights(ap.bitcast(mybir.dt.bfloat16)[:, 0:1])

    pe_boot = [pe_touch(t) for t in (cbA, cbB, cbC, xA, xB)]
    boot_pending = [True]

    ones1 = singles.tile([1, 128], f32)
    nc.vector.memset(ones1, 1.0)
    ident = singles.tile([128, 128], f32)
    make_identity(nc, ident[:, :])

    iota16i = singles.tile([128, 16], i32)
    nc.gpsimd.iota(iota16i, [[1, 16]], base=0, channel_multiplier=0)
    iota16f = singles.tile([128, 16], f32)
    nc.vector.tensor_copy(iota16f, iota16i)
    iota4f = iota16f[:, 0:4]
    siota = singles.tile([128, 1], i32)
    nc.gpsimd.iota(siota, [[0, 1]], base=0, channel_multiplier=1)
    siotaf = singles.tile([128, 1], f32)
    nc.vector.tensor_copy(siotaf, siota)
    junk256 = singles.tile([128, D], f32)
    xsb = singles.tile([128, G, D], f32)
    nc.sync.dma_start(out=xsb, in_=d_x.rearrange("(g p) d -> p g d", p=128))
    identofs = singles.tile([128, 16], i32)
    nc.gpsimd.iota(identofs, [[128, 16]], base=0, channel_multiplier=1)

    # ---- helpers ----
    def sel_matmuls(statA, statB, bias_col):
        vt = vpool.tile([128, NTOT], f32, tag="vt", name="vt")
        for nt in range(8):
            ps = psc.tile([128, 512], f32, tag="ps", name="ps")
            sl = slice(nt * 512, (nt + 1) * 512)
            mmc = nc.tensor.matmul(out=ps, lhsT=_mm(ones1), rhs=_mm(cbC[0:1, sl]),
                                   start=True, stop=False)
            if boot_pending[0]:
                for t in pe_boot:
                    add_dep_helper(mmc.ins, t.ins, reason="pe boot touch order")
                boot_pending[0] = False
            nc.tensor.matmul(out=ps, lhsT=_mm(statA), rhs=_mm(cbA[:, sl]),
                             start=False, stop=False)
            nc.tensor.matmul(out=ps, lhsT=_mm(statB), rhs=_mm(cbB[:, sl]),
                             start=False, stop=True)
            if bias_col is None:
                nc.scalar.copy(out=vt[:, sl], in_=ps)
            else:
                nc.scalar.activation(out=vt[:, sl], in_=ps, func=AF.Identity,
                                     bias=bias_col(nt // 2), scale=1.0)
        return vt

    def top4(vt):
        vals8 = small.tile([128, 8], f32, tag="vals8", name="vals8")
        idx8 = small.tile([128, 8], u32, tag="idx8", name="idx8")
        nc.vector.max(out=vals8, in_=vt)
        nc.vector.max_index(out=idx8, in_max=vals8, in_values=vt)
        return vals8, idx8

    def banned_from_tuples(tuplesf):
        tup_i = small.tile([128, G, 16], i32, tag="tupi", name="tupi")
        nc.vector.tensor_copy(tup_i, tuplesf)
        sh = small.tile([128, G, 16], i32, tag="tupsh", name="tupsh")
        nc.vector.tensor_scalar(out=sh, in0=tup_i, scalar1=10, scalar2=None,
                                op0=ALU.arith_shift_right)
        bb = state.tile([128, G, 16], f32, tag="bbias", name="bbias")
        for sg in range(4):
            eq = small.tile([128, G, 16], f32, tag="beq", name="beq")
            nc.vector.tensor_scalar(out=eq, in0=sh, scalar1=sg, scalar2=None,
                                    op0=ALU.is_equal)
            red = small.tile([128, G, 4], f32, tag="bred", name="bred")
            nc.vector.tensor_reduce(
                out=red,
                in_=eq.rearrange("p g (w k) -> p g w k", w=4),
                axis=AX.X, op=ALU.max)
            nc.vector.tensor_scalar(
                out=bb.rearrange("p g (w k) -> p g w k", w=4)[:, :, :, sg],
                in0=red, scalar1=float(-BIG), scalar2=None, op0=ALU.mult)
        return bb

    def gather_rows(dram_src, off_col, tag):
        """out[p, :] = dram_src[off_col[p], :] ; off_col [128, 1] int32."""
        out = rows.tile([128, D], f32, tag=tag, name=tag)
        nc.gpsimd.indirect_dma_start(
            out=out[:, :], out_offset=None,
            in_=dram_src[:, :],
            in_offset=bass.IndirectOffsetOnAxis(ap=off_col, axis=0))
        return out

    # mselT chunks: stationaries for the selection matmuls
    mselTA = mstate.tile([128, G, 4, 128], f32, tag="mselTA", name="mselTA")
    mselTB = mstate.tile([128, G, 4, 128], f32, tag="mselTB", name="mselTB")

    def update_beams(cand_i32, par_f, it, ssum_in, ssum_out):
        """cand_i32 [128, G, 4] selected codebook ids; par_f [128, G, 4] parent
        beam (float) or None at stage-0; prepares mselT chunks, dm rows,
        ssum_out rows. it = iteration just finished (-1 for stage 0)."""
        po_i = None
        if par_f is not None:
            po = small.tile([128, G, 4], f32, tag="pofs", name="pofs")
            nc.vector.scalar_tensor_tensor(
                out=po, in0=par_f, scalar=128.0,
                in1=siotaf.unsqueeze(1).to_broadcast([128, G, 4]),
                op0=ALU.mult, op1=ALU.add)
            for g in range(G):
                if g:
                    nc.vector.tensor_scalar(out=po[:, g], in0=po[:, g],
                                            scalar1=float(512 * g), scalar2=None,
                                            op0=ALU.add)
            po_i = small.tile([128, G, 4], i32, tag="pofsi", name="pofsi")
            nc.vector.tensor_copy(po_i, po)

        for g in range(G):
            for w in range(4):
                cbs = gather_rows(d_cbr, cand_i32[:, g, w:w + 1], "cbs")
                if par_f is None:
                    ssnew = cbs
                else:
                    sp = gather_rows(ssum_in, po_i[:, g, w:w + 1], "sp")
                    ssnew = rows.tile([128, D], f32, tag="ssnew", name="ssnew")
                    nc.gpsimd.tensor_tensor(out=ssnew, in0=sp, in1=cbs,
                                            op=ALU.add)
                nc.sync.dma_start(
                    out=ssum_out[(g * 4 + w) * 128:(g * 4 + w + 1) * 128, :],
                    in_=ssnew)
                # stationary update: mselT = xTneg + (it+2) * transpose(cbs)
                co = float(it + 2)
                tch = pe_touch(cbs)
                for ch, (mt, xch) in enumerate(((mselTA, xA), (mselTB, xB))):
                    tp = ptr.tile([128, 128], f32, tag="tp", name="tp")
                    tr = nc.tensor.transpose(
                        out=tp[:, :], in_=cbs[:, ch * 128:(ch + 1) * 128],
                        identity=ident[:, :])
                    add_dep_helper(tr.ins, tch.ins, reason="cbs touch order")
                    nc.vector.scalar_tensor_tensor(
                        out=mt[:, g, w], in0=tp, scalar=co,
                        in1=xch[:, g * 128:(g + 1) * 128],
                        op0=ALU.mult, op1=ALU.add)

    # ================= stage 0 =================
    cand16f = state.tile([128, G, 16], f32, tag="cand16f", name="cand16f")
    tuplesf = state.tile([128, G, 16], f32, tag="tuplesf", name="tuplesf0")
    nc.vector.memset(tuplesf, -1.0)
    cand0i = state.tile([128, G, 4], i32, tag="candi", name="cand0i")

    for g in range(G):
        gsl = slice(g * 128, (g + 1) * 128)
        vt = sel_matmuls(xA[:, gsl], xB[:, gsl], None)
        vals8, idx8 = top4(vt)
        nc.vector.tensor_copy(cand0i[:, g, :], idx8[:, 0:4])
        nc.vector.tensor_copy(
            tuplesf.rearrange("p g (w k) -> p g w k", w=4)[:, g, :, 0],
            idx8[:, 0:4])
    bbias = banned_from_tuples(tuplesf)
    if DEBUG_TAPS:
        nc.sync.dma_start(out=dbg["dbg_scand"][:, :, :], in_=cand0i)
    update_beams(cand0i, None, -1, None, d_ssum[0])

    # ================= iterations =================
    for it in range(3):
        for g in range(G):
            for w in range(W):
                vt = sel_matmuls(
                    mselTA[:, g, w], mselTB[:, g, w],
                    bias_col=lambda sg, g=g, w=w:
                        bbias[:, g, w * 4 + sg:w * 4 + sg + 1])
                vals8, idx8 = top4(vt)
                nc.vector.tensor_copy(cand16f[:, g, w * 4:w * 4 + 4], idx8[:, 0:4])

        # ---- d16 = sum((x - (ssum_parent + cb_cand))^2), direct ----
        cand16i = small.tile([128, G, 16], i32, tag="c16i", name="c16i")
        nc.vector.tensor_copy(cand16i, cand16f)
        d16 = small.tile([128, G, 16], f32, tag="d16", name="d16")
        ssum_cur = d_ssum[it % 2]
        for g in range(G):
            for w in range(W):
                spc = gather_rows(ssum_cur,
                                  identofs[:, g * 4 + w:g * 4 + w + 1], "spc")
                for c in range(4):
                    j = w * 4 + c
                    cbg = gather_rows(d_cbr, cand16i[:, g, j:j + 1], "cbg")
                    qt = rows.tile([128, D], f32, tag="qt", name="qt")
                    nc.gpsimd.tensor_tensor(out=qt, in0=spc, in1=cbg, op=ALU.add)
                    et = rows.tile([128, D], f32, tag="et", name="et")
                    nc.vector.scalar_tensor_tensor(
                        out=et, in0=qt, scalar=-1.0, in1=xsb[:, g, :],
                        op0=ALU.mult, op1=ALU.add)
                    nc.scalar.activation(out=junk256, in_=et, func=AF.Square,
                                         accum_out=d16[:, g, j:j + 1])

        # ---- packed keys ----
        nc.vector.tensor_scalar(out=d16, in0=d16, scalar1=128.0, scalar2=511.0,
                                op0=ALU.max, op1=ALU.min)
        # key = ((bits(d) & 0x07FFFFFF) << 4) | j   (bits(d) in [0x43.., 0x44..))
        # kneg = key ^ 0x7FFFFFFF  -> larger kneg = smaller (d, j)
        kbits = small.tile([128, G, 16], i32, tag="kbits", name="kbits")
        nc.vector.tensor_copy(kbits, d16.bitcast(i32))
        keyi = small.tile([128, G, 16], i32, tag="keyi", name="keyi")
        nc.vector.tensor_scalar(out=keyi, in0=kbits,
                                scalar1=0x07FFFFFF, scalar2=4,
                                op0=ALU.bitwise_and, op1=ALU.logical_shift_left)
        nc.vector.tensor_tensor(
            out=keyi, in0=keyi,
            in1=iota16i.unsqueeze(1).to_broadcast([128, G, 16]),
            op=ALU.bitwise_or)
        kneg = small.tile([128, G, 16], i32, tag="kneg", name="kneg")
        nc.vector.tensor_scalar(out=kneg, in0=keyi, scalar1=0x7FFFFFFF,
                                scalar2=None, op0=ALU.bitwise_xor)

        nwin = 4 if it < 2 else 1
        kk = small.tile([128, G, 4], i32, tag="kk", name="kk")
        for g in range(G):
            m8 = small.tile([128, 8], f32, tag="m8", name="m8")
            nc.vector.max(out=m8, in_=kneg[:, g, :].bitcast(f32))
            m8i = small.tile([128, 4], i32, tag="m8i", name="m8i")
            nc.vector.tensor_copy(m8i, m8[:, 0:4].bitcast(i32))
            nc.vector.tensor_scalar(out=kk[:, g, :], in0=m8i,
                                    scalar1=0x7FFFFFFF, scalar2=None,
                                    op0=ALU.bitwise_xor)
        jsel = small.tile([128, G, 4], i32, tag="jsel", name="jsel")
        nc.vector.tensor_scalar(out=jsel, in0=kk, scalar1=15, scalar2=None,
                                op0=ALU.bitwise_and)
        jself = small.tile([128, G, 4], f32, tag="jself", name="jself")
        nc.vector.tensor_copy(jself, jsel)
        pari = small.tile([128, G, 4], i32, tag="pari", name="pari")
        nc.vector.tensor_scalar(out=pari, in0=jsel, scalar1=2, scalar2=3,
                                op0=ALU.logical_shift_right, op1=ALU.bitwise_and)
        parf = small.tile([128, G, 4], f32, tag="parf", name="parf")
        nc.vector.tensor_copy(parf, pari)
        dnew = state.tile([128, G, 4], f32, tag="dprev", name="dnew")
        nc.vector.tensor_scalar(out=dnew.bitcast(i32), in0=kk,
                                scalar1=4, scalar2=0x40000000,
                                op0=ALU.logical_shift_right, op1=ALU.bitwise_or)

        if DEBUG_TAPS:
            nc.sync.dma_start(out=dbg[f"dbg_cand{it}"][:, :, :], in_=cand16f)
            nc.sync.dma_start(out=dbg[f"dbg_jsel{it}"][:, :, :], in_=jsel)
            nc.sync.dma_start(out=dbg[f"dbg_d16_{it}"][:, :, :], in_=d16)
        candself = state.tile([128, G, 4], f32, tag="candsel", name="candself")
        junk16 = small.tile([128, 16], f32, tag="junk16", name="junk16")
        for g in range(G):
            for wn in range(nwin):
                nc.vector.scalar_tensor_tensor(
                    out=junk16, in0=iota16f, scalar=jself[:, g, wn:wn + 1],
                    in1=cand16f[:, g, :], op0=ALU.is_equal, op1=ALU.mult,
                    accum_out=candself[:, g, wn:wn + 1])

        if it < 2:
            dprev = dnew
            ntup = state.tile([128, G, 16], f32, tag="tuplesf", name="ntup")
            nc.vector.memset(ntup, -1.0)
            tv = tuplesf.rearrange("p g (w k) -> p g w k", w=4)
            nv = ntup.rearrange("p g (w k) -> p g w k", w=4)
            junk4 = small.tile([128, 4], f32, tag="junk4", name="junk4")
            for g in range(G):
                for wn in range(4):
                    for k in range(it + 1):
                        nc.vector.scalar_tensor_tensor(
                            out=junk4, in0=iota4f, scalar=parf[:, g, wn:wn + 1],
                            in1=tv[:, g, :, k], op0=ALU.is_equal, op1=ALU.mult,
                            accum_out=nv[:, g, wn:wn + 1, k])
            nc.vector.tensor_copy(nv[:, :, :, it + 1], candself)
            tuplesf = ntup
            bbias = banned_from_tuples(tuplesf)

            cseli = state.tile([128, G, 4], i32, tag="candi", name="cseli")
            nc.vector.tensor_copy(cseli, candself)
            update_beams(cseli, parf, it, d_ssum[it % 2], d_ssum[(it + 1) % 2])
        else:
            bestf = small.tile([128, G, 4], f32, tag="bestf", name="bestf")
            junk4 = small.tile([128, 4], f32, tag="junk4", name="junk4b")
            tv = tuplesf.rearrange("p g (w k) -> p g w k", w=4)
            for g in range(G):
                for k in range(3):
                    nc.vector.scalar_tensor_tensor(
                        out=junk4, in0=iota4f, scalar=parf[:, g, 0:1],
                        in1=tv[:, g, :, k], op0=ALU.is_equal, op1=ALU.mult,
                        accum_out=bestf[:, g, k:k + 1])
            nc.vector.tensor_copy(bestf[:, :, 3], candself[:, :, 0])
            besti = small.tile([128, G, 4], i32, tag="besti", name="besti")
            nc.vector.tensor_copy(besti, bestf)
            nc.sync.dma_start(
                out=d_best.rearrange("(g p) k -> p g k", p=128), in_=besti)

            dbest = small.tile([128, G], f32, tag="dbest", name="dbest")
            nc.vector.tensor_copy(dbest, dnew[:, :, 0])
            nh = small.tile([128, G], f32, tag="nh", name="nh")
            nc.scalar.activation(out=nh, in_=dbest, func=AF.Sqrt)
            nr2 = small.tile([128, G], f32, tag="nr2", name="nr2")
            for g in range(G):
                rvg = vpool.tile([128, D], f32, tag="xg", name="rvg")
                nc.sync.dma_start(out=rvg, in_=d_rv[g * 128:(g + 1) * 128, :])
                nc.scalar.activation(out=junk256, in_=rvg, func=AF.Square,
                                     accum_out=nr2[:, g:g + 1])
            nrs = small.tile([128, G], f32, tag="nrs", name="nrs")
            nc.scalar.activation(out=nrs, in_=nr2, func=AF.Sqrt)
            nrinv = small.tile([128, G], f32, tag="nrinv", name="nrinv")
            nc.vector.reciprocal(out=nrinv, in_=nrs)
            coef = small.tile([128, G], f32, tag="coef", name="coef")
            nc.vector.tensor_tensor(out=coef, in0=nh, in1=nrinv, op=ALU.mult)
            nc.vector.tensor_scalar(out=coef, in0=coef, scalar1=1e-12,
                                    scalar2=None, op0=ALU.add)
            for g in range(G):
                gsl = slice(g * 128, (g + 1) * 128)
                xg = vpool.tile([128, D], f32, tag="xg", name="xg2")
                rvg = vpool.tile([128, D], f32, tag="rvg2", name="rvg2")
                nc.sync.dma_start(out=xg, in_=d_x[gsl, :])
                nc.sync.dma_start(out=rvg, in_=d_rv[gsl, :])
                osb = vpool.tile([128, D], f32, tag="xg", name="osb")
                nc.vector.scalar_tensor_tensor(
                    out=osb, in0=rvg, scalar=coef[:, g:g + 1],
                    in1=xg, op0=ALU.mult, op1=ALU.add)
                nc.sync.dma_start(out=d_nsvq[gsl, :], in_=osb)


def legalize_waits(nc):
    """Hardware instruction formats hold at most ONE sync-wait. Tile sometimes
    attaches several; split the extras onto same-engine NoOps inserted just
    before the instruction (identical semantics on in-order engines)."""
    n_split = 0
    for f in nc.m.functions:
        for b in f.blocks:
            out = []
            for inst in b.instructions:
                si = inst.sync_info
                waits = list(si.on_wait) if si is not None and si.on_wait else []
                if len(waits) > 1:
                    for w in waits[:-1]:
                        nop = mybir.InstNoOp(
                            name=nc.get_next_instruction_name(), ins=[], outs=[])
                        nop.engine = inst.engine
                        nop.sync_info = mybir.SyncInfo(on_wait=[w], on_update=[])
                        out.append(nop)
                        n_split += 1
                    inst.sync_info = mybir.SyncInfo(
                        on_wait=[waits[-1]], on_update=list(si.on_update or []))
                out.append(inst)
            b.instructions[:] = out
    return n_split


# ---------------- host glue ----------------
_NC_CACHE = {}


def _get_nc():
    if "nc" not in _NC_CACHE:
        nc = build_nc()
        legalize_waits(nc)
        _NC_CACHE["nc"] = nc
    return _NC_CACHE["nc"]


def prep_in_maps(x, cb, rv):
    cbTm2 = np.ascontiguousarray((-2.0 * cb.T)).astype(np.float32)
    cb2neg = (-np.sum(cb.astype(np.float32) ** 2, axis=1, dtype=np.float32))
    cb2neg = np.ascontiguousarray(cb2neg[None, :]).astype(np.float32)
    cbrows = np.ascontiguousarray(cb).astype(np.float32)
    in_maps = []
    for c in range(NC):
        sl = slice(c * NSH, (c + 1) * NSH)
        xs = np.ascontiguousarray(x[sl]).astype(np.float32)
        in_maps.append({
            "xTneg": np.ascontiguousarray(-xs.T),
            "xsm": xs,
            "rvsm": np.ascontiguousarray(rv[sl]).astype(np.float32),
            "cbTm2": cbTm2,
            "cb2neg": cb2neg,
            "cbrows": cbrows,
        })
    return in_maps


def _ensure_ntff_hook():
    """bass_utils' trace path imports antenv.axon_hooks, which this image
    lacks. Register an equivalent ctypes-based hook against libaxon_pjrt.so
    (same ABI trn_agent_boot uses) so trace=True yields NTFF profiles."""
    import types
    import ctypes
    import contextlib
    try:
        from antenv.axon_hooks import get_axon_ntff_profile_hook  # noqa: F401
        return
    except ImportError:
        pass
    so_path = "/opt/axon/libaxon_pjrt.so"
    hook = None
    try:
        lib = ctypes.CDLL(so_path)
        if hasattr(lib, "axon_start_nrt_profile"):
            lib.axon_start_nrt_profile.argtypes = [
                ctypes.POINTER(ctypes.c_int64), ctypes.c_size_t]
            lib.axon_start_nrt_profile.restype = ctypes.c_int64
            lib.axon_stop_nrt_profile.argtypes = [ctypes.c_char_p]
            lib.axon_stop_nrt_profile.restype = ctypes.c_int64

            @contextlib.contextmanager
            def _hook(output_dir, device_ids):
                import jax
                jax.devices()
                if device_ids:
                    ids = (ctypes.c_int64 * len(device_ids))(*device_ids)
                    rc = lib.axon_start_nrt_profile(ids, len(device_ids))
                else:
                    rc = lib.axon_start_nrt_profile(None, 0)
                if rc != 0:
                    raise RuntimeError(f"axon_start_nrt_profile rc={rc}")
                try:
                    yield
                finally:
                    n = lib.axon_stop_nrt_profile(str(output_dir).encode())
                    print(f"profile: {n} file(s) written to {output_dir}")
            hook = _hook
    except OSError:
        pass
    import antenv
    mod = types.ModuleType("antenv.axon_hooks")
    mod._hook = hook
    mod.get_axon_ntff_profile_hook = lambda: mod._hook
    mod.set_axon_ntff_profile_hook = lambda h: setattr(mod, "_hook", h)
    sys.modules["antenv.axon_hooks"] = mod
    antenv.axon_hooks = mod


def kernel(input_data, codebooks, random_vector, train_mode, _trace=False):
    x = np.asarray(input_data, np.float32)
    cb = np.asarray(codebooks, np.float32)
    rv = np.asarray(random_vector, np.float32)
    nc = _get_nc()
    in_maps = prep_in_maps(x, cb, rv)
    if _trace:
        _ensure_ntff_hook()
    res = bass_utils.run_bass_kernel_spmd(
        nc, in_maps, core_ids=list(range(NC)), trace=_trace)
    outs = res.results
    nsvq = np.concatenate([outs[c]["nsvq"] for c in range(NC)], axis=0)
    best = np.concatenate([outs[c]["best"] for c in range(NC)], axis=0).astype(np.int64)
    used = np.zeros(S * KCB, np.int32)
    used[best.reshape(-1)] = 1
    used = used.reshape(S, KCB)
    kernel.last_exec_time_ns = getattr(res, "exec_time_ns", None)
    tm = int(np.asarray(train_mode))
    if tm:
        return (nsvq, used)
    quantized = cb[best].sum(axis=1).astype(np.float32)
    return (quantized, used)
